# revision 27
# baseline (speedup 1.0000x reference)
"""Causal single-head attention on 8 Trainium2 NeuronCores.

Problem: x [4, 2048, 1024] fp32; Wq/Wk/Wv [1024, 1024] fp32.
  q/k/v = x @ W*; scores = q k^T / 32 (causal); out = softmax(scores) @ v.

Sharding: 8 cores = 4 batches x 2 roles. Within a batch, the 16
128-row q-blocks are split alternately: role r takes global blocks
g = 2j+r (j = 0..7) — this balances causal attention work between the
pair. Every core projects K~ = x @ (Wk Wq^T) for the full 2048 kv
tokens of its batch and runs causal attention over a padded kv prefix
of 2j+2 blocks per q-block. Both other projections are algebraically
folded away: scores = x_q (Wq Wk^T) x_kv^T, so raw x_q columns serve
directly as Q^T (no Q matmuls); and out = (attn @ x_kv) @ Wv, so the
attn@V contraction runs against raw x rows and Wv is applied to the
[1024, 1024] normalized context of this core's own q rows afterwards
(128 matmuls instead of a 256-matmul V projection of all kv tokens).
Each q-block's tail (normalize -> PE-transpose -> @Wv -> store) is
emitted one block late, software-pipelined under the next block's
score/context matmuls.

The program is SPMD-uniform: role differences live only in the
host-gathered inputs (xq = q-token columns of x^T in q-block order;
xt = full x^T) and in the [128, 256] mask applied to the last two kv
blocks of each padded row ([tril|zeros] for role 0, [ones|tril] for
role 1).

Numerics: all matmuls in bf16 (inputs rounded on host) with fp32
PSUM accumulation; softmax in fp32 without max-subtraction (scores
are O(5), exp can't overflow), normalization deferred to after the
attn@V matmul. End-to-end max-abs error vs the fp32 reference is
~6e-3 of the output scale.
"""

import numpy as np
import ml_dtypes

import concourse.bass as bass
import concourse.bacc as bacc
import concourse.tile as tile
from concourse import mybir
from concourse.bass_utils import run_bass_kernel_spmd
from concourse.masks import make_identity

P = 128
D = 1024          # d_in
E = 1024          # d_out
T = 2048          # seq len
B = 4             # batch
DT = D // P       # 8 d-tiles
ET = E // P       # 8 e-tiles
QB = 8            # q blocks per core
KVB = T // P      # 16 kv blocks
NCORES = 8

FP32 = mybir.dt.float32
BF16 = mybir.dt.bfloat16

_CACHED_NC = None


def _build(do_kv=True, do_attn=True, pmm_bufs=2, pt_bufs=2, pu_bufs=2, work_bufs=2, x_bufs=3):
    nc = bacc.Bacc(None, target_bir_lowering=False)
    # xq: x^T columns of our q tokens (raw features = Q side of the folded
    # score matmul). wk here is the host-folded Wk @ Wq^T.
    xq = nc.dram_tensor("xq", [D, QB * P], BF16, kind="ExternalInput")
    xt = nc.dram_tensor("xt", [D, T], BF16, kind="ExternalInput")
    xn = nc.dram_tensor("xn", [T, D], BF16, kind="ExternalInput")
    wk = nc.dram_tensor("wk", [D, E], BF16, kind="ExternalInput")
    wv = nc.dram_tensor("wv", [D, E], BF16, kind="ExternalInput")
    mask = nc.dram_tensor("mask", [P, 2 * P], BF16, kind="ExternalInput")
    out = nc.dram_tensor("out", [QB * P, E], FP32, kind="ExternalOutput")

    xq_r = xq.rearrange("(dt p) t -> p dt t", p=P)
    xt_r = xt.rearrange("(dt p) t -> p dt t", p=P)

    with tile.TileContext(nc) as tc:
        with (
            tc.tile_pool(name="const", bufs=1) as const,
            tc.tile_pool(name="big", bufs=1) as big,
            tc.tile_pool(name="wpool", bufs=1) as wpool,
            tc.tile_pool(name="xpool", bufs=x_bufs) as xpool,
            tc.tile_pool(name="work", bufs=work_bufs) as work,
            tc.tile_pool(name="small", bufs=8) as small,
            tc.tile_pool(name="pmm", bufs=pmm_bufs, space="PSUM") as pmm,
            tc.tile_pool(name="pt", bufs=pt_bufs, space="PSUM") as pt,
            tc.tile_pool(name="pu", bufs=pu_bufs, space="PSUM") as pu,
        ):
            ident = const.tile([P, P], BF16)
            make_identity(nc, ident[:])
            mask_sb = const.tile([P, 2 * P], BF16)
            nc.sync.dma_start(out=mask_sb[:], in_=mask[:, :])

            KT = big.tile([P, ET, T], BF16)       # K~^T, e-major
            XN = big.tile([P, KVB, D], BF16)      # raw x rows, kv-tile major
            QT = big.tile([P, ET, QB * P], BF16)  # Q^T for our 1024 q rows
            nc.sync.dma_start(out=XN[:], in_=xn.rearrange("(tt p) d -> p tt d", p=P))

            wk_sb = wpool.tile([P, DT, E], BF16, tag="wk")
            nc.sync.dma_start(out=wk_sb[:], in_=wk.rearrange("(dt p) e -> p dt e", p=P))
            wv_sb = wpool.tile([P, DT, E], BF16, tag="wv")
            nc.sync.dma_start(out=wv_sb[:], in_=wv.rearrange("(dt p) e -> p dt e", p=P))

            # Q^T is just the raw q-token features, DMA'd straight in
            nc.sync.dma_start(out=QT[:], in_=xq_r[:, :, :])

            # ---- Phase A: K^T and V projections over the full 2048 kv tokens
            for c in range(T // 512 if do_kv else 0):
                xc = xpool.tile([P, DT, 512], BF16, tag="x")
                nc.sync.dma_start(out=xc[:], in_=xt_r[:, :, 512 * c:512 * (c + 1)])
                for e in range(ET):
                    ps = pmm.tile([P, 512], FP32, tag="mm")
                    for dt in range(DT):
                        nc.tensor.matmul(ps[:], wk_sb[:, dt, e * P:(e + 1) * P],
                                         xc[:, dt, :],
                                         start=(dt == 0), stop=(dt == DT - 1))
                    nc.scalar.copy(KT[:, e, 512 * c:512 * (c + 1)], ps[:])

            # ---- Phase C: attention per q block.
            # The per-block tail (normalize -> transpose -> @Wv -> store) is
            # emitted one block late so its DVE/ACT dependencies resolve
            # while the PE runs the next block's score/context matmuls.
            def emit_tail(U, sums, j):
                recip = small.tile([P, 1], FP32)
                nc.vector.reciprocal(recip[:], sums[:])
                c_sb = work.tile([P, D], BF16, tag="csb")
                for dh in range(2):
                    nc.vector.tensor_scalar_mul(c_sb[:, dh * 512:(dh + 1) * 512],
                                                U[:, dh * 512:(dh + 1) * 512],
                                                recip[:])
                ps_c = pt.tile([P, D], BF16, tag="pt")
                for i in range(DT):
                    nc.tensor.transpose(ps_c[:, i * P:(i + 1) * P],
                                        c_sb[:, i * P:(i + 1) * P], ident[:])
                ct_sb = work.tile([P, D], BF16, tag="ct")
                nc.scalar.copy(ct_sb[:], ps_c[:])
                out_sb = work.tile([P, E], FP32, tag="out")
                for eh in range(2):
                    ps_o = pmm.tile([P, 512], FP32, tag="mm")
                    for dt in range(DT):
                        nc.tensor.matmul(ps_o[:], ct_sb[:, dt * P:(dt + 1) * P],
                                         wv_sb[:, dt, eh * 512:(eh + 1) * 512],
                                         start=(dt == 0), stop=(dt == DT - 1))
                    nc.scalar.copy(out_sb[:, eh * 512:(eh + 1) * 512], ps_o[:])
                nc.sync.dma_start(out=out[j * P:(j + 1) * P, :], in_=out_sb[:])

            pending = None
            for j in range(QB if do_attn else 0):
                n_kb = 2 * j + 2          # padded kv blocks for this q block
                widths = [512] * ((j + 1) // 2) + ([256] if j % 2 == 0 else [])
                sums = small.tile([P, 1], FP32)
                nc.vector.memset(sums[:], 0.0)
                U = pu.tile([P, E], FP32, tag="pu")
                c0 = 0
                for ci, w in enumerate(widths):
                    last = (ci == len(widths) - 1)
                    ps_s = pmm.tile([P, 512], FP32, tag="mm")
                    for et in range(ET):
                        nc.tensor.matmul(ps_s[:, :w], QT[:, et, j * P:(j + 1) * P],
                                         KT[:, et, c0:c0 + w],
                                         start=(et == 0), stop=(et == ET - 1))
                    exps = work.tile([P, 512], BF16, tag="exps")
                    nc.scalar.activation(exps[:, :w], ps_s[:, :w],
                                         mybir.ActivationFunctionType.Exp,
                                         scale=1.0 / 32.0)
                    if last:
                        nc.vector.tensor_mul(exps[:, w - 256:w],
                                             exps[:, w - 256:w], mask_sb[:])
                    csum = small.tile([P, 1], FP32)
                    nc.vector.tensor_reduce(csum[:], exps[:, :w],
                                            axis=mybir.AxisListType.X,
                                            op=mybir.AluOpType.add)
                    nc.vector.tensor_add(sums[:], sums[:], csum[:])

                    ps_t = pt.tile([P, 512], BF16, tag="pt")
                    nblk = w // P
                    for i in range(nblk):
                        nc.tensor.transpose(ps_t[:, i * P:(i + 1) * P],
                                            exps[:, i * P:(i + 1) * P], ident[:])
                    expsT = work.tile([P, 512], BF16, tag="expsT")
                    nc.scalar.copy(expsT[:, :w], ps_t[:, :w])
                    for i in range(nblk):
                        kb = c0 // P + i
                        for dh in range(2):
                            nc.tensor.matmul(U[:, dh * 512:(dh + 1) * 512],
                                             expsT[:, i * P:(i + 1) * P],
                                             XN[:, kb, dh * 512:(dh + 1) * 512],
                                             start=(kb == 0), stop=(kb == n_kb - 1))
                    c0 += w
                if pending is not None:
                    emit_tail(*pending)
                pending = (U, sums, j)
            if pending is not None:
                emit_tail(*pending)

    nc.compile()
    return nc


def _get_nc():
    global _CACHED_NC
    if _CACHED_NC is None:
        _CACHED_NC = _build()
    return _CACHED_NC


def _prep_inputs(x, Wq, Wk, Wv):
    bf = ml_dtypes.bfloat16
    tril = np.tril(np.ones((P, P), np.float32))
    ones = np.ones((P, P), np.float32)
    zeros = np.zeros((P, P), np.float32)
    # fold Wq into the K projection: scores = x_q (Wq Wk^T) x_kv^T, so the
    # kernel projects K~ = x @ (Wk Wq^T) and uses raw x_q as Q.
    wfold = (np.asarray(Wk, np.float64) @ np.asarray(Wq, np.float64).T)
    wk_b = wfold.astype(np.float32).astype(bf)
    wv_b = np.asarray(Wv, np.float32).astype(bf)
    in_maps = []
    for core in range(NCORES):
        b, r = core // 2, core % 2
        xt = np.ascontiguousarray(x[b].T.astype(np.float32)).astype(bf)
        xqc = np.ascontiguousarray(
            xt.reshape(D, KVB, P)[:, r::2, :].reshape(D, QB * P))
        m = (np.concatenate([tril, zeros], axis=1) if r == 0
             else np.concatenate([ones, tril], axis=1)).astype(bf)
        in_maps.append({
            "xq": xqc,
            "xt": xt,
            "xn": np.ascontiguousarray(x[b].astype(np.float32)).astype(bf),
            "wk": wk_b,
            "wv": wv_b,
            "mask": m,
        })
    return in_maps


def _assemble(results, x_shape):
    outp = np.empty(x_shape, np.float32)
    for core in range(NCORES):
        b, r = core // 2, core % 2
        co = results[core]["out"]
        for j in range(QB):
            g = 2 * j + r
            outp[b, g * P:(g + 1) * P, :] = co[j * P:(j + 1) * P, :]
    return outp


def kernel(x, Wq, Wk, Wv):
    assert x.shape == (B, T, D) and Wq.shape == (D, E)
    nc = _get_nc()
    in_maps = _prep_inputs(x, Wq, Wk, Wv)
    res = run_bass_kernel_spmd(nc, in_maps, core_ids=list(range(NCORES)))
    return _assemble(res.results, x.shape)
